# revision 2
# baseline (speedup 1.0000x reference)
"""GatedGraphConv (3-layer, GRU) Bass kernel for 8 Trainium2 NeuronCores.

Strategy:
  - Shard nodes (dst segments) across 8 cores (12500 nodes each).
  - Per layer, each core:
      * gathers h[src] rows for its edges straight from a full copy of h in
        its HBM via dma_gather (int16 indices -> 4 "superblocks" of 25000
        rows each so indices fit in int16),
      * computes segment sums via TensorE matmuls with on-the-fly-built
        one-hot selection matrices S (S[e, j] = ew[e] * (j == dst_rel[e])),
        accumulating in PSUM,  (this also folds in the edge weights)
      * GRU cell entirely in feature-major layout; W_l is folded into W_ih
        on the host (Wi_eff = W_ih @ W_l.T) because segment_sum commutes
        with the linear transform,
      * transposes updated h back to row-major and AllGathers shards so
        every core again holds the full h for the next layer's gather.
"""

import sys
import types
import numpy as np

for _p in ("/opt/trn_rl_repo",):
    if _p not in sys.path:
        sys.path.append(_p)

# ---------------------------------------------------------------------------
# constants (hardcoded problem shape)
# ---------------------------------------------------------------------------
N = 100000          # nodes
D = 128             # feature dim
L = 3               # layers
C = 8               # cores
NC_ = N // C        # nodes per core (12500)
NCP = 12800         # nodes per core, padded to NW*WIN
WIN = 512           # dst nodes per PSUM window
NW = NCP // WIN     # windows per core (25)
G = WIN // 128      # 128-wide subgroups per window (4)
SB = 4              # superblocks of gather table (int16 index limit)
SBROWS = N // SB    # 25000


def _ceil_div(a, b):
    return -(-a // b)


# ---------------------------------------------------------------------------
# host-side planning
# ---------------------------------------------------------------------------
def _plan(edge_index, edge_attr):
    """Build per-core gather indices / S-build scalars / counts."""
    src = np.asarray(edge_index[0], dtype=np.int64)
    dst = np.asarray(edge_index[1], dtype=np.int64)
    ew = np.asarray(edge_attr, dtype=np.float32)

    core = dst // NC_
    dst_local = dst - core * NC_
    sb = src // SBROWS
    src_local = (src - sb * SBROWS).astype(np.int32)
    w = dst_local // WIN
    g = (dst_local % WIN) // 128
    rel = (dst_local % 128).astype(np.float32)

    n_cells = NW * G * SB
    cell = ((w * G + g) * SB + sb).astype(np.int64)
    key = core * n_cells + cell

    order = np.argsort(key, kind="stable")
    key_s = key[order]
    src_s = src_local[order]
    rel_s = rel[order]
    ew_s = ew[order]

    counts = np.bincount(key_s, minlength=C * n_cells).reshape(C, n_cells)
    T = int(_ceil_div(max(1, counts.max()), 128))

    slots_per_cell = T * 128
    idx_all = np.full((C, n_cells, slots_per_cell), -1, dtype=np.int16)
    rel_all = np.zeros((C, n_cells, slots_per_cell), dtype=np.float32)
    ew_all = np.zeros((C, n_cells, slots_per_cell), dtype=np.float32)
    cnt_all = np.maximum(counts, 1).astype(np.int32)

    # fill cells (vectorized): position within cell for each edge
    starts = np.zeros(C * n_cells, dtype=np.int64)
    cc = np.bincount(key_s, minlength=C * n_cells)
    starts[1:] = np.cumsum(cc)[:-1]
    pos = np.arange(len(key_s)) - starts[key_s]

    flat_idx = idx_all.reshape(C * n_cells, slots_per_cell)
    flat_rel = rel_all.reshape(C * n_cells, slots_per_cell)
    flat_ew = ew_all.reshape(C * n_cells, slots_per_cell)
    flat_idx[key_s, pos] = src_s.astype(np.int16)
    flat_rel[key_s, pos] = rel_s
    flat_ew[key_s, pos] = ew_s
    # cells with zero edges get one dummy slot (idx 0, ew 0)
    empty = counts.reshape(-1) == 0
    flat_idx[empty, 0] = 0

    # idx wrapped layout: slot i -> [i % 16, i // 16], replicated to 128 parts
    idx_wr = idx_all.reshape(C, NW, G * SB, slots_per_cell // 16, 16)
    idx_wr = np.ascontiguousarray(np.moveaxis(idx_wr, -1, -2))  # [C,NW,cells,16,slots/16]
    idx_wr = idx_wr.reshape(C, NW, G * SB * 16, slots_per_cell // 16)
    idx_rep = np.tile(idx_wr.reshape(C, NW, G * SB, 16, slots_per_cell // 16),
                      (1, 1, 1, 8, 1))
    idx_rep = idx_rep.reshape(C, NW, G * SB, 128, slots_per_cell // 16)
    idx_rep = np.ascontiguousarray(np.moveaxis(idx_rep, 2, 3))  # [C,NW,128,cells,slots/16]
    idx_rep = idx_rep.reshape(C, NW, 128, G * SB * (slots_per_cell // 16))

    # rel/ew tile-column layout: slot s of cell -> tile t=s//128, part p=s%128
    def tile_cols(a):
        a = a.reshape(C, NW, G * SB, T, 128)
        a = np.moveaxis(a, -1, 2)  # [C, NW, 128, cells, T]
        return np.ascontiguousarray(a).reshape(C, NW, 128, G * SB * T)

    rel_cols = tile_cols(rel_all)
    ew_cols = tile_cols(ew_all)
    cnt_out = cnt_all.reshape(C, 1, n_cells)

    return T, idx_rep, rel_cols, ew_cols, cnt_out


# ---------------------------------------------------------------------------
# device program
# ---------------------------------------------------------------------------
def _build_program(T):
    from contextlib import ExitStack
    import concourse.bass as bass
    import concourse.tile as tile
    from concourse import bacc, mybir

    f32 = mybir.dt.float32
    i16 = mybir.dt.int16
    i32 = mybir.dt.int32
    eq = mybir.AluOpType.is_equal
    mult = mybir.AluOpType.mult
    add = mybir.AluOpType.add

    nc = bacc.Bacc("TRN2", target_bir_lowering=False, debug=False, num_devices=C)

    n_cells = NW * G * SB
    spc = T * 128  # slots per cell

    x_full = nc.dram_tensor("x_full", [N, D], f32, kind="ExternalInput").ap()
    x_ownT = nc.dram_tensor("x_ownT", [D, NCP], f32, kind="ExternalInput").ap()
    idx_dram = nc.dram_tensor("idx_dram", [NW, 128, G * SB * (spc // 16)], i16,
                              kind="ExternalInput").ap()
    rel_dram = nc.dram_tensor("rel_dram", [NW, 128, G * SB * T], f32,
                              kind="ExternalInput").ap()
    ewc_dram = nc.dram_tensor("ewc_dram", [NW, 128, G * SB * T], f32,
                              kind="ExternalInput").ap()
    cnt_dram = nc.dram_tensor("cnt_dram", [1, n_cells], i32, kind="ExternalInput").ap()
    wie_dram = nc.dram_tensor("wie_dram", [128, L * 3 * 128], f32, kind="ExternalInput").ap()
    whh_dram = nc.dram_tensor("whh_dram", [128, 3 * 128], f32, kind="ExternalInput").ap()
    bias_dram = nc.dram_tensor("bias_dram", [128, 4], f32, kind="ExternalInput").ap()
    iota_dram = nc.dram_tensor("iota_dram", [128, 128], f32, kind="ExternalInput").ap()
    ident_dram = nc.dram_tensor("ident_dram", [128, 128], f32, kind="ExternalInput").ap()

    out = nc.dram_tensor("out", [NC_, D], f32, kind="ExternalOutput").ap()

    with tile.TileContext(nc) as tc, ExitStack() as ctx:
        const = ctx.enter_context(tc.tile_pool(name="const", bufs=1))
        dram = ctx.enter_context(tc.tile_pool(name="dram", bufs=1, space="DRAM"))
        idxp = ctx.enter_context(tc.tile_pool(name="idxp", bufs=2))
        sclp = ctx.enter_context(tc.tile_pool(name="sclp", bufs=2))
        sp = ctx.enter_context(tc.tile_pool(name="sp", bufs=4))
        aggps = ctx.enter_context(tc.tile_pool(name="aggps", bufs=2, space="PSUM"))
        grups = ctx.enter_context(tc.tile_pool(name="grups", bufs=1, space="PSUM"))
        tmpp = ctx.enter_context(tc.tile_pool(name="tmpp", bufs=2))
        rowp = ctx.enter_context(tc.tile_pool(name="rowp", bufs=2))

        # resident tensors
        h_sb = const.tile([D, NCP], f32)
        aggp = const.tile([D, NCP], f32)
        iota_sb = const.tile([128, 128], f32)
        ident_sb = const.tile([128, 128], f32)
        wie_sb = const.tile([128, L * 3 * 128], f32)
        whh_sb = const.tile([128, 3 * 128], f32)
        bias_sb = const.tile([128, 4], f32)
        cnt_sb = const.tile([1, n_cells], i32)

        nc.sync.dma_start(h_sb[:], x_ownT[:])
        nc.sync.dma_start(iota_sb[:], iota_dram[:])
        nc.sync.dma_start(ident_sb[:], ident_dram[:])
        nc.sync.dma_start(wie_sb[:], wie_dram[:])
        nc.sync.dma_start(whh_sb[:], whh_dram[:])
        nc.sync.dma_start(bias_sb[:], bias_dram[:])
        nc.sync.dma_start(cnt_sb[:], cnt_dram[:])

        # manually managed msg buffers (memset once; stale data must stay finite)
        NMSG = 4
        msg_bufs = [const.tile([128, spc], f32, name=f"msgbuf{i}") for i in range(NMSG)]
        for mb in msg_bufs:
            nc.vector.memset(mb[:], 0.0)

        h_bounce = [dram.tile([NC_, D], f32, name=f"h_bounce{l}") for l in range(2)]
        h_full = [dram.tile([N, D], f32, name=f"h_full{l}", addr_space="Shared")
                  for l in range(2)]

        NREG = 4
        cnt_regs = [nc.gpsimd.alloc_register(f"cntreg{i}") for i in range(NREG)]

        msg_i = 0
        for l in range(L):
            table = x_full if l == 0 else h_full[l - 1]
            # ---- message aggregation ----
            for w in range(NW):
                idx_w = idxp.tile([128, G * SB * (spc // 16)], i16, tag="idx")
                nc.sync.dma_start(idx_w[:], idx_dram[w])
                rel_w = sclp.tile([128, G * SB * T], f32, tag="rel")
                nc.sync.dma_start(rel_w[:], rel_dram[w])
                ew_w = sclp.tile([128, G * SB * T], f32, tag="ew")
                nc.sync.dma_start(ew_w[:], ewc_dram[w])

                pa = aggps.tile([128, WIN], f32, tag="agg")
                for gq in range(G):
                    for sbi in range(SB):
                        ci = gq * SB + sbi
                        cell = w * (G * SB) + ci
                        msg = msg_bufs[msg_i % NMSG]
                        msg_i += 1
                        cnt = cnt_regs[msg_i % NREG]
                        nc.gpsimd.reg_load(cnt, cnt_sb[0:1, cell:cell + 1])
                        nc.gpsimd.dma_gather(
                            msg.rearrange("p (t f) -> p t f", f=D),
                            table[sbi * SBROWS:(sbi + 1) * SBROWS, :],
                            idx_w[:, ci * (spc // 16):(ci + 1) * (spc // 16)],
                            spc, cnt, D,
                            queue_num=0,
                        )
                        for ti in range(T):
                            col = ci * T + ti
                            S = sp.tile([128, 128], f32, tag="S")
                            nc.vector.tensor_scalar(
                                S[:], iota_sb[:],
                                rel_w[:, col:col + 1], ew_w[:, col:col + 1],
                                op0=eq, op1=mult)
                            nc.tensor.matmul(
                                pa[:, gq * 128:(gq + 1) * 128],
                                lhsT=msg[:, ti * 128:(ti + 1) * 128],
                                rhs=S[:],
                                start=(sbi == 0 and ti == 0),
                                stop=(sbi == SB - 1 and ti == T - 1),
                            )
                nc.vector.tensor_copy(aggp[:, w * WIN:(w + 1) * WIN], pa[:])

            # ---- GRU ----
            for ch in range(NCP // WIN):
                cs = slice(ch * WIN, (ch + 1) * WIN)
                p_r = grups.tile([128, WIN], f32, tag="p_r")
                p_z = grups.tile([128, WIN], f32, tag="p_z")
                p_in = grups.tile([128, WIN], f32, tag="p_in")
                p_hn = grups.tile([128, WIN], f32, tag="p_hn")

                def wie(k):
                    o = (l * 3 + k) * 128
                    return wie_sb[:, o:o + 128]

                def whh(k):
                    return whh_sb[:, k * 128:(k + 1) * 128]

                nc.tensor.matmul(p_r[:], lhsT=wie(0), rhs=aggp[:, cs], start=True, stop=False)
                nc.tensor.matmul(p_r[:], lhsT=whh(0), rhs=h_sb[:, cs], start=False, stop=True)
                nc.tensor.matmul(p_z[:], lhsT=wie(1), rhs=aggp[:, cs], start=True, stop=False)
                nc.tensor.matmul(p_z[:], lhsT=whh(1), rhs=h_sb[:, cs], start=False, stop=True)
                nc.tensor.matmul(p_in[:], lhsT=wie(2), rhs=aggp[:, cs], start=True, stop=True)
                nc.tensor.matmul(p_hn[:], lhsT=whh(2), rhs=h_sb[:, cs], start=True, stop=True)

                r = tmpp.tile([128, WIN], f32, tag="r")
                nc.scalar.activation(r[:], p_r[:], mybir.ActivationFunctionType.Sigmoid,
                                     bias=bias_sb[:, 0:1])
                z = tmpp.tile([128, WIN], f32, tag="z")
                nc.scalar.activation(z[:], p_z[:], mybir.ActivationFunctionType.Sigmoid,
                                     bias=bias_sb[:, 1:2])
                hnb = tmpp.tile([128, WIN], f32, tag="hnb")
                nc.vector.tensor_scalar(hnb[:], p_hn[:], bias_sb[:, 3:4], None, op0=add)
                rt = tmpp.tile([128, WIN], f32, tag="rt")
                nc.vector.tensor_mul(rt[:], r[:], hnb[:])
                s_ = tmpp.tile([128, WIN], f32, tag="s_")
                nc.vector.tensor_add(s_[:], p_in[:], rt[:])
                n_ = tmpp.tile([128, WIN], f32, tag="n_")
                nc.scalar.activation(n_[:], s_[:], mybir.ActivationFunctionType.Tanh,
                                     bias=bias_sb[:, 2:3])
                d_ = tmpp.tile([128, WIN], f32, tag="d_")
                nc.vector.tensor_sub(d_[:], h_sb[:, cs], n_[:])
                zd = tmpp.tile([128, WIN], f32, tag="zd")
                nc.vector.tensor_mul(zd[:], z[:], d_[:])
                nc.vector.tensor_add(h_sb[:, cs], n_[:], zd[:])

                # transpose h chunk to row-major and store
                p_t = grups.tile([128, WIN], f32, tag="p_t")
                for q in range(G):
                    nc.tensor.transpose(
                        p_t[:, q * 128:(q + 1) * 128],
                        h_sb[:, ch * WIN + q * 128: ch * WIN + (q + 1) * 128],
                        ident_sb[:])
                hr = rowp.tile([128, WIN], f32, tag="hr")
                nc.vector.tensor_copy(hr[:], p_t[:])

                dst = h_bounce[l] if l < 2 else out
                r0 = ch * WIN
                hr3 = hr.rearrange("p (q f) -> p q f", f=D)
                if r0 + WIN <= NC_:
                    dview = dst[r0:r0 + WIN, :].rearrange("(q p) f -> p q f", p=128)
                    nc.sync.dma_start(dview, hr3)
                else:
                    # tail chunk: only real rows
                    rem = NC_ - r0
                    nq = rem // 128
                    if nq > 0:
                        dview = dst[r0:r0 + nq * 128, :].rearrange("(q p) f -> p q f", p=128)
                        nc.sync.dma_start(dview, hr3[:, 0:nq, :])
                    rtail = rem - nq * 128
                    if rtail > 0:
                        dview = dst[r0 + nq * 128:r0 + rem, :].rearrange(
                            "(q p) f -> p q f", q=1)
                        nc.sync.dma_start(dview, hr3[0:rtail, nq:nq + 1, :])

            if l < 2:
                nc.gpsimd.collective_compute(
                    "AllGather",
                    mybir.AluOpType.bypass,
                    replica_groups=[list(range(C))],
                    ins=[h_bounce[l].opt()],
                    outs=[h_full[l].opt()],
                )

    nc.compile()
    return nc


# ---------------------------------------------------------------------------
# host wrappers
# ---------------------------------------------------------------------------
def _make_inputs(x, W, W_ih, W_hh, b_ih, b_hh, T, idx_rep, rel_cols, ew_cols, cnt):
    x = np.asarray(x, dtype=np.float32)
    W = np.asarray(W, dtype=np.float32)
    W_ih = np.asarray(W_ih, dtype=np.float32)
    W_hh = np.asarray(W_hh, dtype=np.float32)
    b_ih = np.asarray(b_ih, dtype=np.float32)
    b_hh = np.asarray(b_hh, dtype=np.float32)

    # Wi_eff_l = W_ih @ W_l.T ; lhsT chunk (l,k): Wi_eff_l[k*128:(k+1)*128, :].T
    wie = np.zeros((128, L * 3 * 128), dtype=np.float32)
    for l in range(L):
        wi = W_ih @ W[l].T  # [3D, D]
        for k in range(3):
            wie[:, (l * 3 + k) * 128:(l * 3 + k + 1) * 128] = wi[k * 128:(k + 1) * 128, :].T
    whh = np.zeros((128, 3 * 128), dtype=np.float32)
    for k in range(3):
        whh[:, k * 128:(k + 1) * 128] = W_hh[k * 128:(k + 1) * 128, :].T
    bias = np.zeros((128, 4), dtype=np.float32)
    bias[:, 0] = b_ih[0:128] + b_hh[0:128]
    bias[:, 1] = b_ih[128:256] + b_hh[128:256]
    bias[:, 2] = b_ih[256:384]
    bias[:, 3] = b_hh[256:384]

    iota = np.tile(np.arange(128, dtype=np.float32), (128, 1))
    ident = np.eye(128, dtype=np.float32)

    in_maps = []
    for c in range(C):
        x_ownT = np.zeros((D, NCP), dtype=np.float32)
        x_ownT[:, :NC_] = x[c * NC_:(c + 1) * NC_].T
        in_maps.append({
            "x_full": x,
            "x_ownT": x_ownT,
            "idx_dram": np.ascontiguousarray(idx_rep[c]),
            "rel_dram": np.ascontiguousarray(rel_cols[c]),
            "ewc_dram": np.ascontiguousarray(ew_cols[c]),
            "cnt_dram": np.ascontiguousarray(cnt[c]),
            "wie_dram": wie,
            "whh_dram": whh,
            "bias_dram": bias,
            "iota_dram": np.ascontiguousarray(iota),
            "ident_dram": ident,
        })
    return in_maps


_cache = {}


def _run(x, edge_index, edge_attr, W, W_ih, W_hh, b_ih, b_hh, trace=False):
    from concourse import bass_utils

    T, idx_rep, rel_cols, ew_cols, cnt = _plan(edge_index, edge_attr)
    if T not in _cache:
        _cache[T] = _build_program(T)
    nc = _cache[T]

    in_maps = _make_inputs(x, W, W_ih, W_hh, b_ih, b_hh,
                           T, idx_rep, rel_cols, ew_cols, cnt)
    res = bass_utils.run_bass_kernel_spmd(nc, in_maps, list(range(C)), trace=trace)
    out = np.concatenate([res.results[c]["out"] for c in range(C)], axis=0)
    return out.astype(np.float32), res


def kernel(x, edge_index, edge_attr, W, W_ih, W_hh, b_ih, b_hh):
    out, _ = _run(x, edge_index, edge_attr, W, W_ih, W_hh, b_ih, b_hh)
    return out

